# revision 22
# baseline (speedup 1.0000x reference)
"""Grouped-Query Attention kernel for 8 Trainium2 NeuronCores.

Problem: B=2, T=2048, C=2048, H=16 query heads, KV=4 kv heads, D=128.

Exploited reference properties:
  1. RoPE is applied with seq_len = num_heads, so cos/sin depend only on the
     head index (constant over time). RoPE is a fixed per-head linear map
     folded into wq/bq (and wk/bk) on the host, with the 1/sqrt(D) scale.
  2. The "causal mask" is an ADDITIVE +1 on the lower triangle (torch SDPA
     float-mask semantics). Fully-below-diagonal score tiles get the +1 via
     the exp activation bias; the 4 diagonal tiles get the exact 0/1 tril
     tile added on the PE inside the score accumulation group
     (identity @ tril-tile, start=False/stop=True).

Sharding: core i -> (batch b = i//4, kv-group g = i%4). Each core owns one
KV head and its 4 query heads, computes a partial o_proj over its 512 input
channels; the host sums the 4 partials per batch and adds bo.

Performance design (targets the ~250-280us/core PE roofline):
  - All matmul operands bf16 (1 cycle/row, FWL weight loads, half the SBUF
    and DMA traffic); PSUM accumulation stays fp32.
  - Exp on ACT over 2-bank PSUM groups [128,1024] to amortize access
    latency; softmax colsums via incremental bf16 pair-adds on DVE plus a
    4-matmul ones-reduction on PE (which also broadcasts Z to all
    partitions); 1/Z via reciprocal_approx_fast (the exact reciprocal costs
    3.4us per call).
  - Per-head Z/normalize chain is deferred into the NEXT head's score
    stream so its PE matmuls never stall; o_proj for chunk tch is deferred
    until after head 0 of chunk tch+1 so the end-of-chunk normalize tail
    hides under score matmuls.
  - PSUM budget: scores 2x2 banks, ps_att 2, po 2 = 8 banks.
"""

import numpy as np
import ml_dtypes

import concourse.bass as bass
import concourse.bacc as bacc
import concourse.mybir as mybir
import concourse.tile as tile
from concourse.bass_utils import run_bass_kernel_spmd

F32 = mybir.dt.float32
BF16 = mybir.dt.bfloat16
AF = mybir.ActivationFunctionType
BF16_NP = ml_dtypes.bfloat16

DIM = 2048
H = 16
KV = 4
D = 128          # head dim
G = H // KV      # 4 query heads per kv head
T = 2048
B = 2
NCORES = 8
ROPE_MAX = 2048

_PROGRAM = None


def _build_program():
    nc = bacc.Bacc(None, target_bir_lowering=False, debug=False)

    xT_d = nc.declare_dram_parameter("xT4", [4, 16, 128, 512], BF16, isOutput=False)
    wqT_d = nc.declare_dram_parameter("wqT", [DIM, 512], BF16, isOutput=False)
    wkT_d = nc.declare_dram_parameter("wkT", [DIM, 128], BF16, isOutput=False)
    wvT_d = nc.declare_dram_parameter("wvT", [DIM, 128], BF16, isOutput=False)
    woT_d = nc.declare_dram_parameter("woT", [512, DIM], BF16, isOutput=False)
    bq_d = nc.declare_dram_parameter("bq", [128, 4], F32, isOutput=False)
    bk_d = nc.declare_dram_parameter("bk", [128, 1], F32, isOutput=False)
    bv_d = nc.declare_dram_parameter("bv_col", [128, 1], F32, isOutput=False)
    ident_d = nc.declare_dram_parameter("ident", [128, 128], BF16, isOutput=False)
    m_d = nc.declare_dram_parameter("mtri", [128, 4, 512], F32, isOutput=False)
    ones_d = nc.declare_dram_parameter("ones", [128, 128], BF16, isOutput=False)
    o_d = nc.declare_dram_parameter("o_part", [T, DIM], F32, isOutput=True)

    with tile.TileContext(nc) as tc:
        with tc.tile_pool(name="persist", bufs=1) as persist:
            qT_sb = persist.tile([128, 4, T], BF16)      # [d, h, t]
            kT_sb = persist.tile([128, T], BF16)         # [d, s]
            vT_sb = persist.tile([128, T], BF16)         # [d, s] pre-transpose
            v_sb = persist.tile([128, 16, 128], BF16)    # [s%128, s//128, d]
            m_sb = persist.tile([128, 4, 512], F32)
            ones_sb = persist.tile([128, 128], BF16)
            ident_sb = persist.tile([128, 128], BF16)

            # ---------------- phase 1: q/k/v projections ----------------
            with tc.tile_pool(name="ph1w", bufs=1) as ph1w, \
                 tc.tile_pool(name="xtp", bufs=6) as xtp, \
                 tc.tile_pool(name="ps1", bufs=1, space="PSUM") as ps1, \
                 tc.tile_pool(name="pst", bufs=2, space="PSUM") as ps_t:
                wqT_sb = ph1w.tile([128, 16, 512], BF16)
                wkT_sb = ph1w.tile([128, 16, 128], BF16)
                wvT_sb = ph1w.tile([128, 16, 128], BF16)
                bq_sb = ph1w.tile([128, 4], F32)
                bk_sb = ph1w.tile([128, 1], F32)
                bv_sb = ph1w.tile([128, 1], F32)

                wq_r = wqT_d[:].rearrange("(c p) m -> p c m", p=128)
                wk_r = wkT_d[:].rearrange("(c p) m -> p c m", p=128)
                wv_r = wvT_d[:].rearrange("(c p) m -> p c m", p=128)

                # DMA order tuned so the first matmuls can start ~3us in;
                # the remaining wq chunks and small loads are interleaved
                # with x-tile prefetches inside the loop below.
                wq_chunk = lambda c4: nc.sync.dma_start(
                    out=wqT_sb[:, 4 * c4:4 * c4 + 4, :],
                    in_=wq_r[:, 4 * c4:4 * c4 + 4, :])

                tiles_seq = [(tch, ci) for tch in range(4) for ci in range(16)]
                xt_tiles = {}

                def push_xt(idx):
                    tch, ci = tiles_seq[idx]
                    xt = xtp.tile([128, 512], BF16, tag="xt")
                    nc.sync.dma_start(out=xt, in_=xT_d[tch, ci])
                    xt_tiles[idx] = xt

                wq_chunk(0)
                push_xt(0)
                nc.sync.dma_start(out=wkT_sb, in_=wk_r)
                push_xt(1)
                nc.sync.dma_start(out=wvT_sb, in_=wv_r)
                for i in range(2, 5):
                    push_xt(i)

                for tch in range(4):
                    tsl = slice(tch * 512, (tch + 1) * 512)
                    psq = ps1.tile([128, 4, 512], F32, tag="psq")
                    psk = ps1.tile([128, 512], F32, tag="psk")
                    psv = ps1.tile([128, 512], F32, tag="psv")
                    for ci in range(16):
                        idx = tch * 16 + ci
                        if idx + 5 < 64:
                            push_xt(idx + 5)
                        if idx in (1, 3, 5):
                            wq_chunk((idx + 1) // 2)
                        if idx == 8:
                            # late, non-urgent loads: biases needed at the
                            # first activations (~17us), masks/ones at phase 2
                            nc.sync.dma_start(out=bq_sb, in_=bq_d[:])
                            nc.sync.dma_start(out=bk_sb, in_=bk_d[:])
                            nc.sync.dma_start(out=bv_sb, in_=bv_d[:])
                            nc.sync.dma_start(out=ident_sb, in_=ident_d[:])
                            nc.sync.dma_start(out=m_sb, in_=m_d[:])
                            nc.sync.dma_start(out=ones_sb, in_=ones_d[:])
                        xt = xt_tiles.pop(idx)
                        st = dict(start=(ci == 0), stop=(ci == 15))
                        for do in range(4):
                            nc.tensor.matmul(
                                psq[:, do, :],
                                lhsT=wqT_sb[:, ci, do * 128:(do + 1) * 128],
                                rhs=xt, **st)
                        nc.tensor.matmul(psk, lhsT=wkT_sb[:, ci, :], rhs=xt, **st)
                        nc.tensor.matmul(psv, lhsT=wvT_sb[:, ci, :], rhs=xt, **st)
                    for do in range(4):
                        nc.scalar.activation(
                            qT_sb[:, do, tsl], psq[:, do, :], AF.Identity,
                            bias=bq_sb[:, do:do + 1])
                    nc.scalar.activation(
                        kT_sb[:, tsl], psk, AF.Identity, bias=bk_sb[:, 0:1])
                    nc.scalar.activation(
                        vT_sb[:, tsl], psv, AF.Identity, bias=bv_sb[:, 0:1])
                    # transpose this chunk's vT [d, s] -> v [s, d] via PE
                    for j in range(4):
                        si = tch * 4 + j
                        pst = ps_t.tile([128, 128], BF16, tag="pst")
                        nc.tensor.transpose(
                            pst, vT_sb[:, si * 128:(si + 1) * 128], ident_sb)
                        nc.vector.tensor_copy(v_sb[:, si, :], pst)

            # ---------------- phase 2 + 3: attention and o_proj ----------------
            with tc.tile_pool(name="ph2w", bufs=1) as ph2w, \
                 tc.tile_pool(name="attp", bufs=2) as attp, \
                 tc.tile_pool(name="pTp", bufs=3) as pTp, \
                 tc.tile_pool(name="t8p", bufs=3) as t8p, \
                 tc.tile_pool(name="rp", bufs=3) as rp, \
                 tc.tile_pool(name="otp", bufs=3) as otp, \
                 tc.tile_pool(name="ps_s", bufs=2, space="PSUM") as ps_s, \
                 tc.tile_pool(name="ps_a", bufs=2, space="PSUM") as ps_a, \
                 tc.tile_pool(name="ps_o", bufs=2, space="PSUM") as ps_o:
                woT_sb = ph2w.tile([128, 4, DIM], BF16)
                nc.sync.dma_start(
                    out=woT_sb, in_=woT_d[:].rearrange("(c p) m -> p c m", p=128))

                pending_z = []      # deferred per-head Z/normalize chains
                pending_oproj = []  # deferred per-chunk o_proj blocks

                def make_z_chain(t8, ps_att, attT, h):
                    def z_chain():
                        # ps_o is idle while heads run, and using it keeps the
                        # score pool's 2-group pipelining intact
                        ps_z = ps_o.tile([128, 512], F32, tag="po")
                        nc.tensor.matmul(
                            ps_z, lhsT=ones_sb, rhs=t8[:, 0, :],
                            start=True, stop=True)
                        rinv = rp.tile([128, 512], F32, tag="r")
                        nc.vector.reciprocal_approx_fast(rinv, ps_z)
                        nc.vector.tensor_mul(attT[:, h, :], ps_att, rinv)
                    return z_chain

                def make_oproj(tch, attT):
                    def oproj():
                        for tt in range(4):
                            ttg = tch * 4 + tt
                            for oc in range(4):
                                po = ps_o.tile([128, 512], F32, tag="po")
                                for dok in range(4):
                                    nc.tensor.matmul(
                                        po,
                                        lhsT=attT[:, dok,
                                                  tt * 128:(tt + 1) * 128],
                                        rhs=woT_sb[:, dok,
                                                   oc * 512:(oc + 1) * 512],
                                        start=(dok == 0), stop=(dok == 3))
                                ot = otp.tile([128, 512], F32, tag="ot")
                                # GPSIMD cannot read PSUM; split drains ACT/DVE
                                if (tt * 4 + oc) % 2 == 0:
                                    nc.scalar.activation(ot, po, AF.Copy)
                                else:
                                    nc.vector.tensor_copy(ot, po)
                                nc.sync.dma_start(
                                    out=o_d[ttg * 128:(ttg + 1) * 128,
                                            oc * 512:(oc + 1) * 512],
                                    in_=ot)
                    return oproj

                for tch in range(4):
                    tsl = slice(tch * 512, (tch + 1) * 512)
                    attT = attp.tile([128, 4, 512], BF16, tag="att")

                    for h in range(4):
                        pTt = pTp.tile([128, 16, 512], BF16, tag="pT")
                        t8 = t8p.tile([128, 8, 512], BF16, tag="t8")
                        ps_att = ps_a.tile([128, 512], F32, tag="pa")

                        for gg in range(8):
                            diag = gg in (2 * tch, 2 * tch + 1)
                            ps = ps_s.tile([128, 2, 512], F32, tag="s")
                            for k2 in range(2):
                                si = 2 * gg + k2
                                nc.tensor.matmul(
                                    ps[:, k2, :],
                                    lhsT=kT_sb[:, si * 128:(si + 1) * 128],
                                    rhs=qT_sb[:, h, tsl],
                                    start=True, stop=True)
                            if diag:
                                # +1 on the lower triangle, exact 0/1 tile
                                j = 2 * (gg - 2 * tch)
                                nc.vector.tensor_add(
                                    ps, ps, m_sb[:, j:j + 2, :])
                            if gg == 4 and pending_z:
                                pending_z.pop(0)()
                            if gg >= 1:
                                gp = gg - 1
                                for k2 in range(2):
                                    si = 2 * gp + k2
                                    nc.tensor.matmul(
                                        ps_att,
                                        lhsT=v_sb[:, si, :],
                                        rhs=pTt[:, si, :],
                                        start=(si == 0), stop=(si == 15))
                            bias = 1.0 if gg < 2 * tch else 0.0
                            nc.scalar.activation(
                                pTt[:, 2 * gg:2 * gg + 2, :], ps, AF.Exp,
                                bias=bias)
                            pair_eng = nc.gpsimd if gg in (0, 2, 4, 6) else nc.vector
                            pair_eng.tensor_add(
                                t8[:, gg, :],
                                pTt[:, 2 * gg, :], pTt[:, 2 * gg + 1, :])
                            if gg == 6:
                                nc.vector.tensor_add(
                                    t8[:, 0:2, :], t8[:, 0:2, :], t8[:, 4:6, :])
                        for k2 in range(2):
                            si = 14 + k2
                            nc.tensor.matmul(
                                ps_att,
                                lhsT=v_sb[:, si, :],
                                rhs=pTt[:, si, :],
                                start=(si == 0), stop=(si == 15))
                        nc.vector.tensor_add(
                            t8[:, 2:4, :], t8[:, 2:4, :], t8[:, 6:8, :])
                        nc.vector.tensor_add(
                            t8[:, 0:2, :], t8[:, 0:2, :], t8[:, 2:4, :])
                        nc.vector.tensor_add(
                            t8[:, 0, :], t8[:, 0, :], t8[:, 1, :])

                        pending_z.append(make_z_chain(t8, ps_att, attT, h))

                        if h == 0 and pending_oproj:
                            pending_oproj.pop(0)()

                    pending_oproj.append(make_oproj(tch, attT))

                while pending_z:
                    pending_z.pop(0)()
                while pending_oproj:
                    pending_oproj.pop(0)()
    nc.finalize()
    return nc


def _get_program():
    global _PROGRAM
    if _PROGRAM is None:
        _PROGRAM = _build_program()
    return _PROGRAM


def _rope_cos_sin():
    inv_freq = 1.0 / (10000.0 ** (np.arange(0, D, 2, dtype=np.float64) / D))
    t = np.arange(ROPE_MAX, dtype=np.float64)
    freqs = np.outer(t, inv_freq)                       # [S, D/2]
    emb = np.concatenate([freqs, freqs], axis=-1)       # [S, D]
    return np.cos(emb).astype(np.float32), np.sin(emb).astype(np.float32)


def _fold_rope(w, b, nheads, scale):
    """Fold per-head RoPE (position index = head index) into weight rows.

    w: [nheads*D, C], b: [nheads*D]. Returns rotated (and scaled) copies.
    """
    cos, sin = _rope_cos_sin()
    w = w.reshape(nheads, D, -1)
    b = b.reshape(nheads, D)
    c = cos[:nheads][:, :, None]          # [nheads, D, 1]
    s = sin[:nheads][:, :, None]
    w_rot = np.empty_like(w)
    hD = D // 2
    w_rot[:, :hD] = w[:, :hD] * c[:, :hD] - w[:, hD:] * s[:, :hD]
    w_rot[:, hD:] = w[:, hD:] * c[:, hD:] + w[:, :hD] * s[:, hD:]
    cb = cos[:nheads]
    sb = sin[:nheads]
    b_rot = np.empty_like(b)
    b_rot[:, :hD] = b[:, :hD] * cb[:, :hD] - b[:, hD:] * sb[:, :hD]
    b_rot[:, hD:] = b[:, hD:] * cb[:, hD:] + b[:, :hD] * sb[:, hD:]
    return (w_rot.reshape(nheads * D, -1) * scale).astype(np.float32), \
           (b_rot.reshape(nheads * D) * scale).astype(np.float32)


def _bf16(a):
    return np.ascontiguousarray(a).astype(BF16_NP)


def _host_inputs(x, wq, bq, wk, bk, wv, bv, wo, bo):
    """Build the per-core input maps."""
    scale = float(D) ** -0.5
    wq_r, bq_r = _fold_rope(wq.astype(np.float32), bq.astype(np.float32), H, scale)
    wk_r, bk_r = _fold_rope(wk.astype(np.float32), bk.astype(np.float32), KV, 1.0)

    # diagonal mask tiles: mtri[p, j, t'] = 1 if j*128 + p <= t' else 0
    p_idx = np.arange(128)[:, None, None]
    j_idx = np.arange(4)[None, :, None]
    t_idx = np.arange(512)[None, None, :]
    mtri = ((j_idx * 128 + p_idx) <= t_idx).astype(np.float32)

    # x[b].T tiled as [tch, ci, 128, 512] contiguous bf16
    xT4 = []
    for b in range(B):
        xt = np.ascontiguousarray(x[b].T.astype(np.float32))
        xt = xt.reshape(16, 128, 4, 512).transpose(2, 0, 1, 3)
        xT4.append(_bf16(xt))

    mtri_f32 = np.ascontiguousarray(mtri)
    ones_bf = _bf16(np.ones((128, 128), np.float32))
    ident_bf = _bf16(np.eye(128, dtype=np.float32))

    in_maps = []
    for core in range(NCORES):
        b, g = divmod(core, G)
        qs = slice(512 * g, 512 * (g + 1))
        ks = slice(128 * g, 128 * (g + 1))
        in_maps.append({
            "xT4": xT4[b],
            "wqT": _bf16(wq_r[qs].T),
            "wkT": _bf16(wk_r[ks].T),
            "wvT": _bf16(wv[ks].astype(np.float32).T),
            "woT": _bf16(wo[:, qs].astype(np.float32).T),
            "bq": np.ascontiguousarray(bq_r[qs].reshape(4, 128).T),
            "bk": np.ascontiguousarray(bk_r[ks].reshape(128, 1)),
            "bv_col": np.ascontiguousarray(
                bv[ks].astype(np.float32).reshape(128, 1)),
            "ident": ident_bf,
            "mtri": mtri_f32,
            "ones": ones_bf,
        })
    return in_maps


def run_cores(inputs, trace=False, **kw):
    nc = _get_program()
    in_maps = _host_inputs(**inputs)
    res = run_bass_kernel_spmd(nc, in_maps, list(range(NCORES)), trace=trace, **kw)
    return res


def kernel(**inputs) -> np.ndarray:
    res = run_cores(inputs)
    bo = inputs["bo"].astype(np.float32)
    out = np.empty((B, T, DIM), dtype=np.float32)
    for b in range(B):
        acc = res.results[b * G + 0]["o_part"].astype(np.float32).copy()
        for g in range(1, G):
            acc += res.results[b * G + g]["o_part"]
        out[b] = acc + bo
    return out


# revision 23
# speedup vs baseline: 1.2033x; 1.2033x over previous
"""Grouped-Query Attention kernel for 8 Trainium2 NeuronCores.

Problem: B=2, T=2048, C=2048, H=16 query heads, KV=4 kv heads, D=128.

Exploited reference properties:
  1. RoPE is applied with seq_len = num_heads, so cos/sin depend only on the
     head index (constant over time). RoPE is a fixed per-head linear map
     folded into wq/bq (and wk/bk) on the host, with the 1/sqrt(D) scale.
  2. The "causal mask" is an ADDITIVE +1 on the lower triangle (torch SDPA
     float-mask semantics). Fully-below-diagonal score tiles get the +1 via
     the exp activation bias; the 4 diagonal tiles get the exact 0/1 tril
     tile added on the PE inside the score accumulation group
     (identity @ tril-tile, start=False/stop=True).

Sharding: core i -> (batch b = i//4, kv-group g = i%4). Each core owns one
KV head and its 4 query heads, computes a partial o_proj over its 512 input
channels; the host sums the 4 partials per batch and adds bo.

Performance design (targets the ~250-280us/core PE roofline):
  - All matmul operands bf16 (1 cycle/row, FWL weight loads, half the SBUF
    and DMA traffic); PSUM accumulation stays fp32.
  - Exp on ACT over 2-bank PSUM groups [128,1024] to amortize access
    latency; softmax colsums via incremental bf16 pair-adds on DVE plus a
    4-matmul ones-reduction on PE (which also broadcasts Z to all
    partitions); 1/Z via reciprocal_approx_fast (the exact reciprocal costs
    3.4us per call).
  - Per-head Z/normalize chain is deferred into the NEXT head's score
    stream so its PE matmuls never stall; o_proj for chunk tch is deferred
    until after head 0 of chunk tch+1 so the end-of-chunk normalize tail
    hides under score matmuls.
  - PSUM budget: scores 2x2 banks, ps_att 2, po 2 = 8 banks.
"""

import numpy as np
import ml_dtypes

import concourse.bass as bass
import concourse.bacc as bacc
import concourse.mybir as mybir
import concourse.tile as tile
from concourse.bass_utils import run_bass_kernel_spmd

F32 = mybir.dt.float32
BF16 = mybir.dt.bfloat16
AF = mybir.ActivationFunctionType
BF16_NP = ml_dtypes.bfloat16

DIM = 2048
H = 16
KV = 4
D = 128          # head dim
G = H // KV      # 4 query heads per kv head
T = 2048
B = 2
NCORES = 8
ROPE_MAX = 2048

_PROGRAM = None


def _build_program():
    nc = bacc.Bacc(None, target_bir_lowering=False, debug=False)

    xT_d = nc.declare_dram_parameter("xT4", [4, 16, 128, 512], BF16, isOutput=False)
    wqT_d = nc.declare_dram_parameter("wqT", [DIM, 512], BF16, isOutput=False)
    wkT_d = nc.declare_dram_parameter("wkT", [DIM, 128], BF16, isOutput=False)
    wvT_d = nc.declare_dram_parameter("wvT", [DIM, 128], BF16, isOutput=False)
    woT_d = nc.declare_dram_parameter("woT", [512, DIM], BF16, isOutput=False)
    bq_d = nc.declare_dram_parameter("bq", [128, 4], F32, isOutput=False)
    bk_d = nc.declare_dram_parameter("bk", [128, 1], F32, isOutput=False)
    bv_d = nc.declare_dram_parameter("bv_col", [128, 1], F32, isOutput=False)
    ident_d = nc.declare_dram_parameter("ident", [128, 128], BF16, isOutput=False)
    m_d = nc.declare_dram_parameter("mtri", [128, 4, 512], F32, isOutput=False)
    ones_d = nc.declare_dram_parameter("ones", [128, 128], BF16, isOutput=False)
    o_d = nc.declare_dram_parameter("o_part", [T, DIM], F32, isOutput=True)

    with tile.TileContext(nc) as tc:
        with tc.tile_pool(name="persist", bufs=1) as persist:
            qT_sb = persist.tile([128, 4, T], BF16)      # [d, h, t]
            kT_sb = persist.tile([128, T], BF16)         # [d, s]
            vT_sb = persist.tile([128, T], BF16)         # [d, s] pre-transpose
            v_sb = persist.tile([128, 16, 128], BF16)    # [s%128, s//128, d]
            m_sb = persist.tile([128, 4, 512], F32)
            ones_sb = persist.tile([128, 128], BF16)
            ident_sb = persist.tile([128, 128], BF16)

            # ---------------- phase 1: q/k/v projections ----------------
            with tc.tile_pool(name="ph1w", bufs=1) as ph1w, \
                 tc.tile_pool(name="xtp", bufs=6) as xtp, \
                 tc.tile_pool(name="ps1", bufs=1, space="PSUM") as ps1, \
                 tc.tile_pool(name="pst", bufs=2, space="PSUM") as ps_t:
                wqT_sb = ph1w.tile([128, 16, 512], BF16)
                wkT_sb = ph1w.tile([128, 16, 128], BF16)
                wvT_sb = ph1w.tile([128, 16, 128], BF16)
                bq_sb = ph1w.tile([128, 4], F32)
                bk_sb = ph1w.tile([128, 1], F32)
                bv_sb = ph1w.tile([128, 1], F32)

                wq_r = wqT_d[:].rearrange("(c p) m -> p c m", p=128)
                wk_r = wkT_d[:].rearrange("(c p) m -> p c m", p=128)
                wv_r = wvT_d[:].rearrange("(c p) m -> p c m", p=128)

                # DMA order tuned so the first matmuls can start ~3us in;
                # the remaining wq chunks and small loads are interleaved
                # with x-tile prefetches inside the loop below.
                wq_chunk = lambda c4: nc.sync.dma_start(
                    out=wqT_sb[:, 4 * c4:4 * c4 + 4, :],
                    in_=wq_r[:, 4 * c4:4 * c4 + 4, :])

                tiles_seq = [(tch, ci) for tch in range(4) for ci in range(16)]
                xt_tiles = {}

                def push_xt(idx):
                    tch, ci = tiles_seq[idx]
                    xt = xtp.tile([128, 512], BF16, tag="xt")
                    nc.sync.dma_start(out=xt, in_=xT_d[tch, ci])
                    xt_tiles[idx] = xt

                wq_chunk(0)
                push_xt(0)
                nc.sync.dma_start(out=wkT_sb, in_=wk_r)
                push_xt(1)
                nc.sync.dma_start(out=wvT_sb, in_=wv_r)
                for i in range(2, 5):
                    push_xt(i)

                for tch in range(4):
                    tsl = slice(tch * 512, (tch + 1) * 512)
                    psq = ps1.tile([128, 4, 512], F32, tag="psq")
                    psk = ps1.tile([128, 512], F32, tag="psk")
                    psv = ps1.tile([128, 512], F32, tag="psv")
                    for ci in range(16):
                        idx = tch * 16 + ci
                        if idx + 5 < 64:
                            push_xt(idx + 5)
                        if idx in (1, 3, 5):
                            wq_chunk((idx + 1) // 2)
                        if idx == 8:
                            # late, non-urgent loads: biases needed at the
                            # first activations (~17us), masks/ones at phase 2
                            nc.sync.dma_start(out=bq_sb, in_=bq_d[:])
                            nc.sync.dma_start(out=bk_sb, in_=bk_d[:])
                            nc.sync.dma_start(out=bv_sb, in_=bv_d[:])
                            nc.sync.dma_start(out=ident_sb, in_=ident_d[:])
                            nc.sync.dma_start(out=m_sb, in_=m_d[:])
                            nc.sync.dma_start(out=ones_sb, in_=ones_d[:])
                        xt = xt_tiles.pop(idx)
                        st = dict(start=(ci == 0), stop=(ci == 15))
                        for do in range(4):
                            nc.tensor.matmul(
                                psq[:, do, :],
                                lhsT=wqT_sb[:, ci, do * 128:(do + 1) * 128],
                                rhs=xt, **st)
                        nc.tensor.matmul(psk, lhsT=wkT_sb[:, ci, :], rhs=xt, **st)
                        nc.tensor.matmul(psv, lhsT=wvT_sb[:, ci, :], rhs=xt, **st)
                    for do in range(4):
                        nc.scalar.activation(
                            qT_sb[:, do, tsl], psq[:, do, :], AF.Identity,
                            bias=bq_sb[:, do:do + 1])
                    nc.scalar.activation(
                        kT_sb[:, tsl], psk, AF.Identity, bias=bk_sb[:, 0:1])
                    nc.scalar.activation(
                        vT_sb[:, tsl], psv, AF.Identity, bias=bv_sb[:, 0:1])
                    # transpose this chunk's vT [d, s] -> v [s, d] via PE
                    for j in range(4):
                        si = tch * 4 + j
                        pst = ps_t.tile([128, 128], BF16, tag="pst")
                        nc.tensor.transpose(
                            pst, vT_sb[:, si * 128:(si + 1) * 128], ident_sb)
                        nc.vector.tensor_copy(v_sb[:, si, :], pst)

            # ---------------- phase 2 + 3: attention and o_proj ----------------
            with tc.tile_pool(name="ph2w", bufs=1) as ph2w, \
                 tc.tile_pool(name="attp", bufs=2) as attp, \
                 tc.tile_pool(name="pTp", bufs=3) as pTp, \
                 tc.tile_pool(name="t8p", bufs=3) as t8p, \
                 tc.tile_pool(name="rp", bufs=3) as rp, \
                 tc.tile_pool(name="otp", bufs=3) as otp, \
                 tc.tile_pool(name="ps_s", bufs=2, space="PSUM") as ps_s, \
                 tc.tile_pool(name="ps_a", bufs=2, space="PSUM") as ps_a, \
                 tc.tile_pool(name="ps_o", bufs=2, space="PSUM") as ps_o:
                woT_sb = ph2w.tile([128, 4, DIM], BF16)
                nc.sync.dma_start(
                    out=woT_sb, in_=woT_d[:].rearrange("(c p) m -> p c m", p=128))

                pending_z = []      # deferred per-head Z/normalize chains
                pending_oproj = []  # deferred per-chunk o_proj blocks

                def make_z_chain(t8, ps_att, attT, h):
                    def z_chain():
                        # ps_o is idle while heads run, and using it keeps the
                        # score pool's 2-group pipelining intact
                        ps_z = ps_o.tile([128, 512], F32, tag="po")
                        nc.tensor.matmul(
                            ps_z, lhsT=ones_sb, rhs=t8[:, 0, :],
                            start=True, stop=True)
                        rinv = rp.tile([128, 512], F32, tag="r")
                        nc.vector.reciprocal_approx_fast(rinv, ps_z)
                        nc.vector.tensor_mul(attT[:, h, :], ps_att, rinv)
                    return z_chain

                def make_oproj(tch, attT):
                    def oproj():
                        for tt in range(4):
                            ttg = tch * 4 + tt
                            for oc in range(4):
                                po = ps_o.tile([128, 512], F32, tag="po")
                                for dok in range(4):
                                    nc.tensor.matmul(
                                        po,
                                        lhsT=attT[:, dok,
                                                  tt * 128:(tt + 1) * 128],
                                        rhs=woT_sb[:, dok,
                                                   oc * 512:(oc + 1) * 512],
                                        start=(dok == 0), stop=(dok == 3))
                                ot = otp.tile([128, 512], F32, tag="ot")
                                # GPSIMD cannot read PSUM; split drains ACT/DVE
                                if (tt * 4 + oc) % 2 == 0:
                                    nc.scalar.activation(ot, po, AF.Copy)
                                else:
                                    nc.vector.tensor_copy(ot, po)
                                nc.sync.dma_start(
                                    out=o_d[ttg * 128:(ttg + 1) * 128,
                                            oc * 512:(oc + 1) * 512],
                                    in_=ot)
                    return oproj

                for tch in range(4):
                    tsl = slice(tch * 512, (tch + 1) * 512)
                    attT = attp.tile([128, 4, 512], BF16, tag="att")

                    for h in range(4):
                        pTt = pTp.tile([128, 16, 512], BF16, tag="pT")
                        t8 = t8p.tile([128, 8, 512], BF16, tag="t8")
                        ps_att = ps_a.tile([128, 512], F32, tag="pa")

                        for gg in range(8):
                            diag = gg in (2 * tch, 2 * tch + 1)
                            ps = ps_s.tile([128, 2, 512], F32, tag="s")
                            for k2 in range(2):
                                si = 2 * gg + k2
                                nc.tensor.matmul(
                                    ps[:, k2, :],
                                    lhsT=kT_sb[:, si * 128:(si + 1) * 128],
                                    rhs=qT_sb[:, h, tsl],
                                    start=True, stop=True)
                            if diag:
                                # +1 on the lower triangle, exact 0/1 tile
                                j = 2 * (gg - 2 * tch)
                                nc.vector.tensor_add(
                                    ps, ps, m_sb[:, j:j + 2, :])
                            if gg == 4 and pending_z:
                                pending_z.pop(0)()
                            if gg >= 1:
                                gp = gg - 1
                                for k2 in range(2):
                                    si = 2 * gp + k2
                                    nc.tensor.matmul(
                                        ps_att,
                                        lhsT=v_sb[:, si, :],
                                        rhs=pTt[:, si, :],
                                        start=(si == 0), stop=(si == 15))
                            bias = 1.0 if gg < 2 * tch else 0.0
                            nc.scalar.activation(
                                pTt[:, 2 * gg:2 * gg + 2, :], ps, AF.Exp,
                                bias=bias)
                            pair_eng = nc.gpsimd if gg in (0, 2, 4) else nc.vector
                            pair_eng.tensor_add(
                                t8[:, gg, :],
                                pTt[:, 2 * gg, :], pTt[:, 2 * gg + 1, :])
                            if gg == 6:
                                nc.vector.tensor_add(
                                    t8[:, 0:2, :], t8[:, 0:2, :], t8[:, 4:6, :])
                        for k2 in range(2):
                            si = 14 + k2
                            nc.tensor.matmul(
                                ps_att,
                                lhsT=v_sb[:, si, :],
                                rhs=pTt[:, si, :],
                                start=(si == 0), stop=(si == 15))
                        nc.vector.tensor_add(
                            t8[:, 2:4, :], t8[:, 2:4, :], t8[:, 6:8, :])
                        nc.vector.tensor_add(
                            t8[:, 0:2, :], t8[:, 0:2, :], t8[:, 2:4, :])
                        nc.vector.tensor_add(
                            t8[:, 0, :], t8[:, 0, :], t8[:, 1, :])

                        pending_z.append(make_z_chain(t8, ps_att, attT, h))

                        if h == 0 and pending_oproj:
                            pending_oproj.pop(0)()

                    pending_oproj.append(make_oproj(tch, attT))

                while pending_z:
                    pending_z.pop(0)()
                while pending_oproj:
                    pending_oproj.pop(0)()
    nc.finalize()
    return nc


def _get_program():
    global _PROGRAM
    if _PROGRAM is None:
        _PROGRAM = _build_program()
    return _PROGRAM


def _rope_cos_sin():
    inv_freq = 1.0 / (10000.0 ** (np.arange(0, D, 2, dtype=np.float64) / D))
    t = np.arange(ROPE_MAX, dtype=np.float64)
    freqs = np.outer(t, inv_freq)                       # [S, D/2]
    emb = np.concatenate([freqs, freqs], axis=-1)       # [S, D]
    return np.cos(emb).astype(np.float32), np.sin(emb).astype(np.float32)


def _fold_rope(w, b, nheads, scale):
    """Fold per-head RoPE (position index = head index) into weight rows.

    w: [nheads*D, C], b: [nheads*D]. Returns rotated (and scaled) copies.
    """
    cos, sin = _rope_cos_sin()
    w = w.reshape(nheads, D, -1)
    b = b.reshape(nheads, D)
    c = cos[:nheads][:, :, None]          # [nheads, D, 1]
    s = sin[:nheads][:, :, None]
    w_rot = np.empty_like(w)
    hD = D // 2
    w_rot[:, :hD] = w[:, :hD] * c[:, :hD] - w[:, hD:] * s[:, :hD]
    w_rot[:, hD:] = w[:, hD:] * c[:, hD:] + w[:, :hD] * s[:, hD:]
    cb = cos[:nheads]
    sb = sin[:nheads]
    b_rot = np.empty_like(b)
    b_rot[:, :hD] = b[:, :hD] * cb[:, :hD] - b[:, hD:] * sb[:, :hD]
    b_rot[:, hD:] = b[:, hD:] * cb[:, hD:] + b[:, :hD] * sb[:, hD:]
    return (w_rot.reshape(nheads * D, -1) * scale).astype(np.float32), \
           (b_rot.reshape(nheads * D) * scale).astype(np.float32)


def _bf16(a):
    return np.ascontiguousarray(a).astype(BF16_NP)


def _host_inputs(x, wq, bq, wk, bk, wv, bv, wo, bo):
    """Build the per-core input maps."""
    scale = float(D) ** -0.5
    wq_r, bq_r = _fold_rope(wq.astype(np.float32), bq.astype(np.float32), H, scale)
    wk_r, bk_r = _fold_rope(wk.astype(np.float32), bk.astype(np.float32), KV, 1.0)

    # diagonal mask tiles: mtri[p, j, t'] = 1 if j*128 + p <= t' else 0
    p_idx = np.arange(128)[:, None, None]
    j_idx = np.arange(4)[None, :, None]
    t_idx = np.arange(512)[None, None, :]
    mtri = ((j_idx * 128 + p_idx) <= t_idx).astype(np.float32)

    # x[b].T tiled as [tch, ci, 128, 512] contiguous bf16
    xT4 = []
    for b in range(B):
        xt = np.ascontiguousarray(x[b].T.astype(np.float32))
        xt = xt.reshape(16, 128, 4, 512).transpose(2, 0, 1, 3)
        xT4.append(_bf16(xt))

    mtri_f32 = np.ascontiguousarray(mtri)
    ones_bf = _bf16(np.ones((128, 128), np.float32))
    ident_bf = _bf16(np.eye(128, dtype=np.float32))

    in_maps = []
    for core in range(NCORES):
        b, g = divmod(core, G)
        qs = slice(512 * g, 512 * (g + 1))
        ks = slice(128 * g, 128 * (g + 1))
        in_maps.append({
            "xT4": xT4[b],
            "wqT": _bf16(wq_r[qs].T),
            "wkT": _bf16(wk_r[ks].T),
            "wvT": _bf16(wv[ks].astype(np.float32).T),
            "woT": _bf16(wo[:, qs].astype(np.float32).T),
            "bq": np.ascontiguousarray(bq_r[qs].reshape(4, 128).T),
            "bk": np.ascontiguousarray(bk_r[ks].reshape(128, 1)),
            "bv_col": np.ascontiguousarray(
                bv[ks].astype(np.float32).reshape(128, 1)),
            "ident": ident_bf,
            "mtri": mtri_f32,
            "ones": ones_bf,
        })
    return in_maps


def run_cores(inputs, trace=False, **kw):
    nc = _get_program()
    in_maps = _host_inputs(**inputs)
    res = run_bass_kernel_spmd(nc, in_maps, list(range(NCORES)), trace=trace, **kw)
    return res


def kernel(**inputs) -> np.ndarray:
    res = run_cores(inputs)
    bo = inputs["bo"].astype(np.float32)
    out = np.empty((B, T, DIM), dtype=np.float32)
    for b in range(B):
        acc = res.results[b * G + 0]["o_part"].astype(np.float32).copy()
        for g in range(1, G):
            acc += res.results[b * G + g]["o_part"]
        out[b] = acc + bo
    return out


# revision 27
# speedup vs baseline: 1.3071x; 1.0863x over previous
"""Grouped-Query Attention kernel for 8 Trainium2 NeuronCores.

Problem: B=2, T=2048, C=2048, H=16 query heads, KV=4 kv heads, D=128.

Exploited reference properties:
  1. RoPE is applied with seq_len = num_heads, so cos/sin depend only on the
     head index (constant over time). RoPE is a fixed per-head linear map
     folded into wq/bq (and wk/bk) on the host, with the 1/sqrt(D) scale.
  2. The "causal mask" is an ADDITIVE +1 on the lower triangle (torch SDPA
     float-mask semantics). Fully-below-diagonal score tiles get the +1 via
     the exp activation bias; the 4 diagonal tiles get the exact 0/1 tril
     tile added on the PE inside the score accumulation group
     (identity @ tril-tile, start=False/stop=True).

Sharding: core i -> (batch b = i//4, kv-group g = i%4). Each core owns one
KV head and its 4 query heads, computes a partial o_proj over its 512 input
channels; the host sums the 4 partials per batch and adds bo.

Performance design (targets the ~250-280us/core PE roofline):
  - All matmul operands bf16 (1 cycle/row, FWL weight loads, half the SBUF
    and DMA traffic); PSUM accumulation stays fp32.
  - Exp on ACT over 2-bank PSUM groups [128,1024] to amortize access
    latency; softmax colsums via incremental bf16 pair-adds on DVE plus a
    4-matmul ones-reduction on PE (which also broadcasts Z to all
    partitions); 1/Z via reciprocal_approx_fast (the exact reciprocal costs
    3.4us per call).
  - Per-head Z/normalize chain is deferred into the NEXT head's score
    stream so its PE matmuls never stall; o_proj for chunk tch is deferred
    until after head 0 of chunk tch+1 so the end-of-chunk normalize tail
    hides under score matmuls.
  - PSUM budget: scores 2x2 banks, ps_att 2, po 2 = 8 banks.
"""

import numpy as np
import ml_dtypes

import concourse.bass as bass
import concourse.bacc as bacc
import concourse.mybir as mybir
import concourse.tile as tile
from concourse.bass_utils import run_bass_kernel_spmd

F32 = mybir.dt.float32
BF16 = mybir.dt.bfloat16
AF = mybir.ActivationFunctionType
BF16_NP = ml_dtypes.bfloat16

DIM = 2048
H = 16
KV = 4
D = 128          # head dim
G = H // KV      # 4 query heads per kv head
T = 2048
B = 2
NCORES = 8
ROPE_MAX = 2048

_PROGRAM = None


def _build_program():
    nc = bacc.Bacc(None, target_bir_lowering=False, debug=False)

    xT_d = nc.declare_dram_parameter("xT4", [4, 16, 128, 512], BF16, isOutput=False)
    wqT_d = nc.declare_dram_parameter("wqT", [DIM, 512], BF16, isOutput=False)
    wkT_d = nc.declare_dram_parameter("wkT", [DIM, 128], BF16, isOutput=False)
    wvT_d = nc.declare_dram_parameter("wvT", [DIM, 128], BF16, isOutput=False)
    woT_d = nc.declare_dram_parameter("woT", [512, DIM], BF16, isOutput=False)
    bq_d = nc.declare_dram_parameter("bq", [128, 4], F32, isOutput=False)
    bk_d = nc.declare_dram_parameter("bk", [128, 1], F32, isOutput=False)
    bv_d = nc.declare_dram_parameter("bv_col", [128, 1], F32, isOutput=False)
    ident_d = nc.declare_dram_parameter("ident", [128, 128], BF16, isOutput=False)
    m_d = nc.declare_dram_parameter("mtri", [128, 4, 512], F32, isOutput=False)
    ones_d = nc.declare_dram_parameter("ones", [128, 128], BF16, isOutput=False)
    o_d = nc.declare_dram_parameter("o_part", [T, DIM], F32, isOutput=True)

    with tile.TileContext(nc) as tc:
        with tc.tile_pool(name="persist", bufs=1) as persist:
            qT_sb = persist.tile([128, 4, T], BF16)      # [d, h, t]
            kT_sb = persist.tile([128, T], BF16)         # [d, s]
            vT_sb = persist.tile([128, T], BF16)         # [d, s] pre-transpose
            v_sb = persist.tile([128, 16, 128], BF16)    # [s%128, s//128, d]
            m_sb = persist.tile([128, 4, 512], F32)
            ones_sb = persist.tile([128, 128], BF16)
            ident_sb = persist.tile([128, 128], BF16)

            # ---------------- phase 1: q/k/v projections ----------------
            with tc.tile_pool(name="ph1w", bufs=1) as ph1w, \
                 tc.tile_pool(name="xtp", bufs=6) as xtp, \
                 tc.tile_pool(name="ps1", bufs=1, space="PSUM") as ps1, \
                 tc.tile_pool(name="pst", bufs=2, space="PSUM") as ps_t:
                wqT_sb = ph1w.tile([128, 16, 512], BF16)
                wkT_sb = ph1w.tile([128, 16, 128], BF16)
                wvT_sb = ph1w.tile([128, 16, 128], BF16)
                bq_sb = ph1w.tile([128, 4], F32)
                bk_sb = ph1w.tile([128, 1], F32)
                bv_sb = ph1w.tile([128, 1], F32)

                wq_r = wqT_d[:].rearrange("(c p) m -> p c m", p=128)
                wk_r = wkT_d[:].rearrange("(c p) m -> p c m", p=128)
                wv_r = wvT_d[:].rearrange("(c p) m -> p c m", p=128)

                # DMA order tuned so the first matmuls can start ~3us in;
                # the remaining wq chunks and small loads are interleaved
                # with x-tile prefetches inside the loop below.
                wq_chunk = lambda c4: nc.sync.dma_start(
                    out=wqT_sb[:, 4 * c4:4 * c4 + 4, :],
                    in_=wq_r[:, 4 * c4:4 * c4 + 4, :])

                tiles_seq = [(tch, ci) for tch in range(4) for ci in range(16)]
                xt_tiles = {}

                def push_xt(idx):
                    tch, ci = tiles_seq[idx]
                    xt = xtp.tile([128, 512], BF16, tag="xt")
                    nc.sync.dma_start(out=xt, in_=xT_d[tch, ci])
                    xt_tiles[idx] = xt

                wq_chunk(0)
                push_xt(0)
                nc.sync.dma_start(out=wkT_sb, in_=wk_r)
                push_xt(1)
                nc.sync.dma_start(out=wvT_sb, in_=wv_r)
                for i in range(2, 5):
                    push_xt(i)

                for tch in range(4):
                    tsl = slice(tch * 512, (tch + 1) * 512)
                    psq = ps1.tile([128, 4, 512], F32, tag="psq")
                    psk = ps1.tile([128, 512], F32, tag="psk")
                    psv = ps1.tile([128, 512], F32, tag="psv")
                    for ci in range(16):
                        idx = tch * 16 + ci
                        if idx + 5 < 64:
                            push_xt(idx + 5)
                        if idx in (1, 3, 5):
                            wq_chunk((idx + 1) // 2)
                        if idx == 8:
                            # late, non-urgent loads: biases needed at the
                            # first activations (~17us), masks/ones at phase 2
                            nc.sync.dma_start(out=bq_sb, in_=bq_d[:])
                            nc.sync.dma_start(out=bk_sb, in_=bk_d[:])
                            nc.sync.dma_start(out=bv_sb, in_=bv_d[:])
                            nc.sync.dma_start(out=ident_sb, in_=ident_d[:])
                            nc.sync.dma_start(out=m_sb, in_=m_d[:])
                            nc.sync.dma_start(out=ones_sb, in_=ones_d[:])
                        xt = xt_tiles.pop(idx)
                        st = dict(start=(ci == 0), stop=(ci == 15))
                        for do in range(4):
                            nc.tensor.matmul(
                                psq[:, do, :],
                                lhsT=wqT_sb[:, ci, do * 128:(do + 1) * 128],
                                rhs=xt, **st)
                        nc.tensor.matmul(psk, lhsT=wkT_sb[:, ci, :], rhs=xt, **st)
                        nc.tensor.matmul(psv, lhsT=wvT_sb[:, ci, :], rhs=xt, **st)
                    for do in range(4):
                        nc.scalar.activation(
                            qT_sb[:, do, tsl], psq[:, do, :], AF.Identity,
                            bias=bq_sb[:, do:do + 1])
                    nc.scalar.activation(
                        kT_sb[:, tsl], psk, AF.Identity, bias=bk_sb[:, 0:1])
                    nc.scalar.activation(
                        vT_sb[:, tsl], psv, AF.Identity, bias=bv_sb[:, 0:1])
                    # transpose this chunk's vT [d, s] -> v [s, d] via PE
                    for j in range(4):
                        si = tch * 4 + j
                        pst = ps_t.tile([128, 128], BF16, tag="pst")
                        nc.tensor.transpose(
                            pst, vT_sb[:, si * 128:(si + 1) * 128], ident_sb)
                        nc.vector.tensor_copy(v_sb[:, si, :], pst)

            # ---------------- phase 2 + 3: attention and o_proj ----------------
            with tc.tile_pool(name="ph2w", bufs=1) as ph2w, \
                 tc.tile_pool(name="attp", bufs=2) as attp, \
                 tc.tile_pool(name="pTp", bufs=3) as pTp, \
                 tc.tile_pool(name="t8p", bufs=3) as t8p, \
                 tc.tile_pool(name="rp", bufs=3) as rp, \
                 tc.tile_pool(name="otp", bufs=3) as otp, \
                 tc.tile_pool(name="ps_s", bufs=2, space="PSUM") as ps_s, \
                 tc.tile_pool(name="ps_a", bufs=2, space="PSUM") as ps_a, \
                 tc.tile_pool(name="ps_o", bufs=2, space="PSUM") as ps_o:
                woT_sb = ph2w.tile([128, 4, DIM], BF16)
                nc.sync.dma_start(
                    out=woT_sb, in_=woT_d[:].rearrange("(c p) m -> p c m", p=128))

                pending_z = []      # deferred per-head Z/normalize chains
                pending_oproj = []  # deferred per-chunk o_proj blocks

                def make_z_chain(t8, ps_att, attT, h):
                    def z_chain():
                        # ps_o is idle while heads run, and using it keeps the
                        # score pool's 2-group pipelining intact
                        ps_z = ps_o.tile([128, 512], F32, tag="po")
                        for q in range(2):
                            nc.tensor.matmul(
                                ps_z, lhsT=ones_sb, rhs=t8[:, q, :],
                                start=(q == 0), stop=(q == 1))
                        rinv = rp.tile([128, 512], F32, tag="r")
                        nc.vector.reciprocal_approx_fast(rinv, ps_z)
                        nc.vector.tensor_mul(attT[:, h, :], ps_att, rinv)
                    return z_chain

                def make_oproj_block(tch, attT, tt):
                    def oproj_block():
                        ttg = tch * 4 + tt
                        for oc in range(4):
                            po = ps_o.tile([128, 512], F32, tag="po")
                            for dok in range(4):
                                nc.tensor.matmul(
                                    po,
                                    lhsT=attT[:, dok,
                                              tt * 128:(tt + 1) * 128],
                                    rhs=woT_sb[:, dok,
                                               oc * 512:(oc + 1) * 512],
                                    start=(dok == 0), stop=(dok == 3))
                            ot = otp.tile([128, 512], F32, tag="ot")
                            # GPSIMD cannot read PSUM; split drains ACT/DVE
                            if oc % 2 == 0:
                                nc.scalar.activation(ot, po, AF.Copy)
                            else:
                                nc.vector.tensor_copy(ot, po)
                            nc.sync.dma_start(
                                out=o_d[ttg * 128:(ttg + 1) * 128,
                                        oc * 512:(oc + 1) * 512],
                                in_=ot)
                    return oproj_block

                for tch in range(4):
                    tsl = slice(tch * 512, (tch + 1) * 512)
                    attT = attp.tile([128, 4, 512], BF16, tag="att")

                    for h in range(4):
                        pTt = pTp.tile([128, 16, 512], BF16, tag="pT")
                        t8 = t8p.tile([128, 8, 512], BF16, tag="t8")
                        ps_att = ps_a.tile([128, 512], F32, tag="pa")

                        for gg in range(8):
                            diag = gg in (2 * tch, 2 * tch + 1)
                            ps = ps_s.tile([128, 2, 512], F32, tag="s")
                            for k2 in range(2):
                                si = 2 * gg + k2
                                nc.tensor.matmul(
                                    ps[:, k2, :],
                                    lhsT=kT_sb[:, si * 128:(si + 1) * 128],
                                    rhs=qT_sb[:, h, tsl],
                                    start=True, stop=True)
                            if diag:
                                # +1 on the lower triangle, exact 0/1 tile
                                j = 2 * (gg - 2 * tch)
                                nc.vector.tensor_add(
                                    ps, ps, m_sb[:, j:j + 2, :])
                            if gg == 4 and pending_z:
                                pending_z.pop(0)()
                            if gg >= 1:
                                gp = gg - 1
                                for k2 in range(2):
                                    si = 2 * gp + k2
                                    nc.tensor.matmul(
                                        ps_att,
                                        lhsT=v_sb[:, si, :],
                                        rhs=pTt[:, si, :],
                                        start=(si == 0), stop=(si == 15))
                            bias = 1.0 if gg < 2 * tch else 0.0
                            nc.scalar.activation(
                                pTt[:, 2 * gg:2 * gg + 2, :], ps, AF.Exp,
                                bias=bias)
                            pair_eng = nc.gpsimd if gg in (0, 2, 4) else nc.vector
                            pair_eng.tensor_add(
                                t8[:, gg, :],
                                pTt[:, 2 * gg, :], pTt[:, 2 * gg + 1, :])
                            if gg == 6:
                                nc.vector.tensor_add(
                                    t8[:, 0:2, :], t8[:, 0:2, :], t8[:, 4:6, :])
                        for k2 in range(2):
                            si = 14 + k2
                            nc.tensor.matmul(
                                ps_att,
                                lhsT=v_sb[:, si, :],
                                rhs=pTt[:, si, :],
                                start=(si == 0), stop=(si == 15))
                        nc.vector.tensor_add(
                            t8[:, 2:4, :], t8[:, 2:4, :], t8[:, 6:8, :])
                        nc.vector.tensor_add(
                            t8[:, 0:2, :], t8[:, 0:2, :], t8[:, 2:4, :])

                        pending_z.append(make_z_chain(t8, ps_att, attT, h))

                        if pending_oproj:
                            pending_oproj.pop(0)()

                    for tt in range(4):
                        pending_oproj.append(make_oproj_block(tch, attT, tt))

                while pending_z:
                    pending_z.pop(0)()
                while pending_oproj:
                    pending_oproj.pop(0)()
    nc.finalize()
    return nc


def _get_program():
    global _PROGRAM
    if _PROGRAM is None:
        _PROGRAM = _build_program()
    return _PROGRAM


def _rope_cos_sin():
    inv_freq = 1.0 / (10000.0 ** (np.arange(0, D, 2, dtype=np.float64) / D))
    t = np.arange(ROPE_MAX, dtype=np.float64)
    freqs = np.outer(t, inv_freq)                       # [S, D/2]
    emb = np.concatenate([freqs, freqs], axis=-1)       # [S, D]
    return np.cos(emb).astype(np.float32), np.sin(emb).astype(np.float32)


def _fold_rope(w, b, nheads, scale):
    """Fold per-head RoPE (position index = head index) into weight rows.

    w: [nheads*D, C], b: [nheads*D]. Returns rotated (and scaled) copies.
    """
    cos, sin = _rope_cos_sin()
    w = w.reshape(nheads, D, -1)
    b = b.reshape(nheads, D)
    c = cos[:nheads][:, :, None]          # [nheads, D, 1]
    s = sin[:nheads][:, :, None]
    w_rot = np.empty_like(w)
    hD = D // 2
    w_rot[:, :hD] = w[:, :hD] * c[:, :hD] - w[:, hD:] * s[:, :hD]
    w_rot[:, hD:] = w[:, hD:] * c[:, hD:] + w[:, :hD] * s[:, hD:]
    cb = cos[:nheads]
    sb = sin[:nheads]
    b_rot = np.empty_like(b)
    b_rot[:, :hD] = b[:, :hD] * cb[:, :hD] - b[:, hD:] * sb[:, :hD]
    b_rot[:, hD:] = b[:, hD:] * cb[:, hD:] + b[:, :hD] * sb[:, hD:]
    return (w_rot.reshape(nheads * D, -1) * scale).astype(np.float32), \
           (b_rot.reshape(nheads * D) * scale).astype(np.float32)


def _bf16(a):
    return np.ascontiguousarray(a).astype(BF16_NP)


def _host_inputs(x, wq, bq, wk, bk, wv, bv, wo, bo):
    """Build the per-core input maps."""
    scale = float(D) ** -0.5
    wq_r, bq_r = _fold_rope(wq.astype(np.float32), bq.astype(np.float32), H, scale)
    wk_r, bk_r = _fold_rope(wk.astype(np.float32), bk.astype(np.float32), KV, 1.0)

    # diagonal mask tiles: mtri[p, j, t'] = 1 if j*128 + p <= t' else 0
    p_idx = np.arange(128)[:, None, None]
    j_idx = np.arange(4)[None, :, None]
    t_idx = np.arange(512)[None, None, :]
    mtri = ((j_idx * 128 + p_idx) <= t_idx).astype(np.float32)

    # x[b].T tiled as [tch, ci, 128, 512] contiguous bf16
    xT4 = []
    for b in range(B):
        xt = np.ascontiguousarray(x[b].T.astype(np.float32))
        xt = xt.reshape(16, 128, 4, 512).transpose(2, 0, 1, 3)
        xT4.append(_bf16(xt))

    mtri_f32 = np.ascontiguousarray(mtri)
    ones_bf = _bf16(np.ones((128, 128), np.float32))
    ident_bf = _bf16(np.eye(128, dtype=np.float32))

    in_maps = []
    for core in range(NCORES):
        b, g = divmod(core, G)
        qs = slice(512 * g, 512 * (g + 1))
        ks = slice(128 * g, 128 * (g + 1))
        in_maps.append({
            "xT4": xT4[b],
            "wqT": _bf16(wq_r[qs].T),
            "wkT": _bf16(wk_r[ks].T),
            "wvT": _bf16(wv[ks].astype(np.float32).T),
            "woT": _bf16(wo[:, qs].astype(np.float32).T),
            "bq": np.ascontiguousarray(bq_r[qs].reshape(4, 128).T),
            "bk": np.ascontiguousarray(bk_r[ks].reshape(128, 1)),
            "bv_col": np.ascontiguousarray(
                bv[ks].astype(np.float32).reshape(128, 1)),
            "ident": ident_bf,
            "mtri": mtri_f32,
            "ones": ones_bf,
        })
    return in_maps


def run_cores(inputs, trace=False, **kw):
    nc = _get_program()
    in_maps = _host_inputs(**inputs)
    res = run_bass_kernel_spmd(nc, in_maps, list(range(NCORES)), trace=trace, **kw)
    return res


def kernel(**inputs) -> np.ndarray:
    res = run_cores(inputs)
    bo = inputs["bo"].astype(np.float32)
    out = np.empty((B, T, DIM), dtype=np.float32)
    for b in range(B):
        acc = res.results[b * G + 0]["o_part"].astype(np.float32).copy()
        for g in range(1, G):
            acc += res.results[b * G + g]["o_part"]
        out[b] = acc + bo
    return out
